# revision 19
# baseline (speedup 1.0000x reference)
"""PriorLSTM Trainium2 kernel (8 NeuronCores, SPMD).

Model: BatchNorm1d(IN) -> 16-layer LSTM(H=128) -> Linear(H->OUT) -> max over T
       -> + prior logits.   B=16, T=32, IN=52686, OUT=2976.

Strategy:
  Phase A (tensor-parallel on IN): each core owns 6656 channels (padded).
    BN folded to per-channel scale/shift; big GEMM gx0.T[g,tok] accumulated
    over 52 K-chunks of 128 channels; one AllReduce of the [128,2048]
    partial gives every core the full layer-0 input projection.
  Phase B (replicated, zero collectives): every core runs the whole
    16-layer LSTM over all 16 batch samples. The per-step gate matmul
    [128x128]x[128,16] is weight-load dominated, so 16 samples cost the
    same as 2 -- replication buys out all cross-core traffic.
    States transposed: h.T/c.T = [128 h-dim, 16 batch]; weights are
    pre-transposed host-side so gates come out as gates.T [128,16] per
    gate block.  tanh via sigmoid identity (g rows pre-scaled by 2).
  Phase C: output projection + temporal max-pool; b_out and prior logits
    folded host-side into one additive constant.  Identical on all
    cores; core 0's output is returned.

The whole body (A+B+C) can be repeated `repeat` times inside one NEFF;
every pass recomputes identical values (used for slope-based timing of
true device execution, since the axon tunnel RTT ~85ms swamps wall
clock).  Tokens are time-major: tok = t*16 + b.
"""

import numpy as np

B, T, IN, H, L, OUT = 16, 32, 52686, 128, 16, 2976
EPS = 1e-5
NC = 8
INL = 6656          # channels per core (padded)
NCH = INL // 128    # 52 K-chunks per core
INP = INL * NC      # 53248
TOK = B * T         # 512
OUTP = 3072         # padded OUT


def build_kernel(sim=False, repeat=1, arep=1):
    import concourse.bass as bass
    import concourse.bacc as bacc
    import concourse.mybir as mybir
    import concourse.tile as tile

    f32 = mybir.dt.float32
    Alu = mybir.AluOpType
    Act = mybir.ActivationFunctionType

    nc = bacc.Bacc(None, num_devices=1 if sim else NC)

    # ---------------- inputs ------------------------------------------------
    xT = nc.dram_tensor("xT", [INL, TOK], f32, kind="ExternalInput")
    w0T = nc.dram_tensor("w0T", [INL, 512], f32, kind="ExternalInput")
    gam = nc.dram_tensor("gam", [128, NCH], f32, kind="ExternalInput")
    bet = nc.dram_tensor("bet", [128, NCH], f32, kind="ExternalInput")
    # wih_all[p, (l-1)*512 + m]: W_ih[l].T for layers 1..15 (g rows x2)
    wih_all = nc.dram_tensor("wih_all", [128, (L - 1) * 512], f32,
                             kind="ExternalInput")
    # whh_all[p, l*512 + m]: W_hh[l].T for layers 0..15 (g rows x2)
    whh_all = nc.dram_tensor("whh_all", [128, L * 512], f32,
                             kind="ExternalInput")
    # bias_all[p, l*4 + gb] = (b_ih+b_hh)[l][gb*128+p] (g block x2)
    bias_all = nc.dram_tensor("bias_all", [128, 4 * L], f32,
                              kind="ExternalInput")
    woutT = nc.dram_tensor("woutT", [128, OUTP], f32, kind="ExternalInput")
    padd = nc.dram_tensor("padd", [16, OUTP], f32, kind="ExternalInput")

    outp = nc.dram_tensor("outp", [16, OUTP], f32, kind="ExternalOutput")

    with tile.TileContext(nc) as tc:
        with (
            tc.tile_pool(name="big", bufs=1) as big,
            tc.tile_pool(name="wstream", bufs=3) as wst,
            tc.tile_pool(name="small", bufs=2) as small,
            tc.tile_pool(name="ew", bufs=3) as ew,
            tc.tile_pool(name="dram", bufs=2, space="DRAM") as dpool,
        ):
            # persistent loads (once per NEFF)
            sums = big.tile([128, NCH], f32, tag="sums")
            sumsq = big.tile([128, NCH], f32, tag="sumsq")
            gams = big.tile([128, NCH], f32, tag="gams")
            bets = big.tile([128, NCH], f32, tag="bets")
            nc.sync.dma_start(out=gams[:], in_=gam[:])
            nc.sync.dma_start(out=bets[:], in_=bet[:])

            wih_s = big.tile([128, (L - 1) * 512], f32, tag="wih")
            whh_s = big.tile([128, L * 512], f32, tag="whh")
            ball_s = big.tile([128, 4 * L], f32, tag="ball")
            nc.sync.dma_start(out=wih_s[:], in_=wih_all[:])
            nc.sync.dma_start(out=whh_s[:], in_=whh_all[:])
            nc.sync.dma_start(out=ball_s[:], in_=bias_all[:])

            gx0 = big.tile([128, 4 * TOK], f32, tag="gx0")
            # channel ch = p*NCH + c: per-partition reads are contiguous
            xview = xT.rearrange("(p c) t -> p c t", c=NCH)

            for r in range(repeat):
                # ---------------- phase A ----------------------------------
              for _a in range(arep):
                with tc.tile_pool(name=f"xtsp{r}_{_a}", bufs=1) as xtsp:
                    SLAB = 13
                    xts_l = [xtsp.tile([128, SLAB * TOK], f32, tag=f"xts{s}",
                                       name=f"xts{r}_{_a}_{s}") for s in range(4)]

                    def xchunk(ch):
                        return xts_l[ch // SLAB][
                            :, (ch % SLAB) * TOK:(ch % SLAB + 1) * TOK]

                    for s in range(4):
                        nc.sync.dma_start(
                            out=xts_l[s][:].rearrange(
                                "p (c t) -> p c t", t=TOK),
                            in_=xview[:, s * SLAB:(s + 1) * SLAB, :],
                        )
                    for ch in range(NCH):
                        xc = xchunk(ch)
                        scr = small.tile([128, TOK], f32, tag="scr")
                        nc.vector.tensor_reduce(
                            sums[:, ch:ch + 1], xc,
                            mybir.AxisListType.X, Alu.add,
                        )
                        nc.vector.scalar_tensor_tensor(
                            out=scr[:], in0=xc, scalar=1.0, in1=xc,
                            op0=Alu.mult, op1=Alu.mult,
                            accum_out=sumsq[:, ch:ch + 1],
                        )
                    mean = big.tile([128, NCH], f32, tag="mean")
                    var = big.tile([128, NCH], f32, tag="var")
                    sd = big.tile([128, NCH], f32, tag="sd")
                    r0 = big.tile([128, NCH], f32, tag="r0")
                    t1 = big.tile([128, NCH], f32, tag="t1")
                    scl = big.tile([128, NCH], f32, tag="scl")
                    b2 = big.tile([128, NCH], f32, tag="b2")
                    nc.vector.tensor_scalar_mul(mean[:], sums[:], 1.0 / TOK)
                    nc.vector.tensor_scalar_mul(var[:], sumsq[:], 1.0 / TOK)
                    nc.vector.scalar_tensor_tensor(
                        out=t1[:], in0=mean[:], scalar=-1.0, in1=mean[:],
                        op0=Alu.mult, op1=Alu.mult)
                    nc.vector.tensor_tensor(var[:], var[:], t1[:], Alu.add)
                    nc.vector.tensor_scalar_add(var[:], var[:], EPS)
                    nc.scalar.activation(sd[:], var[:], Act.Sqrt)
                    nc.vector.reciprocal(r0[:], sd[:])
                    nc.vector.tensor_tensor(t1[:], r0[:], r0[:], Alu.mult)
                    nc.vector.tensor_tensor(t1[:], t1[:], var[:], Alu.mult)
                    nc.vector.tensor_scalar(
                        out=t1[:], in0=t1[:], scalar1=-0.5, scalar2=1.5,
                        op0=Alu.mult, op1=Alu.add)
                    nc.vector.tensor_tensor(r0[:], r0[:], t1[:], Alu.mult)
                    nc.vector.tensor_tensor(scl[:], r0[:], gams[:], Alu.mult)
                    nc.vector.scalar_tensor_tensor(
                        out=b2[:], in0=mean[:], scalar=-1.0, in1=scl[:],
                        op0=Alu.mult, op1=Alu.mult)
                    nc.vector.tensor_tensor(b2[:], b2[:], bets[:], Alu.add)

                    with tc.tile_pool(name=f"psgx{r}_{_a}", bufs=1,
                                      space="PSUM") as psgx:
                        gx = [psgx.tile([128, TOK], f32, name=f"gx{r}_{_a}_{gb}",
                                        tag=f"gx{gb}") for gb in range(4)]
                        for ch in range(NCH):
                            xc = xchunk(ch)
                            nc.vector.tensor_scalar(
                                out=xc, in0=xc,
                                scalar1=scl[:, ch:ch + 1],
                                scalar2=b2[:, ch:ch + 1],
                                op0=Alu.mult, op1=Alu.add)
                            wc = wst.tile([128, 512], f32, tag="wc")
                            nc.sync.dma_start(
                                out=wc[:], in_=w0T[ch * 128:(ch + 1) * 128, :])
                            for gb in range(4):
                                nc.tensor.matmul(
                                    gx[gb][:], wc[:, gb * 128:(gb + 1) * 128],
                                    xc,
                                    start=(ch == 0), stop=(ch == NCH - 1),
                                    skip_group_check=True)

                        for gb in range(4):
                            nc.vector.tensor_copy(
                                gx0[:, gb * TOK:(gb + 1) * TOK], gx[gb][:])

                arin = dpool.tile([128, 4 * TOK], f32, tag="arin")
                arout = dpool.tile([128, 4 * TOK], f32, tag="arout")
                nc.gpsimd.dma_start(out=arin[:], in_=gx0[:])
                if sim:
                    nc.gpsimd.dma_start(out=arout[:], in_=arin[:])
                else:
                    nc.gpsimd.collective_compute(
                        "AllReduce", Alu.add,
                        ins=[arin[:].opt()], outs=[arout[:].opt()],
                        replica_groups=[list(range(NC))],
                    )
                nc.sync.dma_start(out=gx0[:], in_=arout[:])

                # ---------------- phase B ----------------------------------
                cst = big.tile([128, 16], f32, tag="cst")
                hall_prev = None
                with tc.tile_pool(name=f"hp{r}", bufs=2) as hp:
                  with (
                    tc.tile_pool(name=f"gxp{r}", bufs=2) as gxp,
                    tc.tile_pool(name=f"psg2{r}", bufs=1,
                                 space="PSUM") as psg2,
                    tc.tile_pool(name=f"psz{r}", bufs=4, space="PSUM") as psz,
                  ):
                    for l in range(L):
                        gxl = gxp.tile([128, 4 * TOK], f32, tag="gxl")
                        if l == 0:
                            for gb in range(4):
                                nc.vector.tensor_scalar_add(
                                    gxl[:, gb * TOK:(gb + 1) * TOK],
                                    gx0[:, gb * TOK:(gb + 1) * TOK],
                                    ball_s[:, l * 4 + gb:l * 4 + gb + 1])
                        else:
                            for gb in range(4):
                                pg = psg2.tile([128, TOK], f32, tag=f"pg{gb}")
                                nc.tensor.matmul(
                                    pg[:],
                                    wih_s[:, (l - 1) * 512 + gb * 128:
                                          (l - 1) * 512 + (gb + 1) * 128],
                                    hall_prev[:],
                                    start=True, stop=True,
                                    skip_group_check=True)
                                nc.vector.tensor_scalar_add(
                                    gxl[:, gb * TOK:(gb + 1) * TOK], pg[:],
                                    ball_s[:, l * 4 + gb:l * 4 + gb + 1])
                        gxv = gxl[:, :].rearrange("p (g t b) -> p g t b",
                                                  g=4, b=16)

                        hall = hp.tile([128, TOK], f32, tag="hall")

                        # gate blocks host-reordered to (i, f, o, g)
                        for t in range(T):
                            sg = ew.tile([128, 64], f32, tag="sg")
                            if t == 0:
                                nc.scalar.activation(
                                    sg[:, 0:48], gxv[:, 0:3, 0, :],
                                    Act.Sigmoid)
                                nc.scalar.activation(
                                    sg[:, 48:64], gxv[:, 3:4, 0, :],
                                    Act.Tanh)
                            else:
                                zt = psz.tile([128, 64], f32, tag="zt")
                                # ACT preloads gx+bias; matmuls accumulate
                                nc.scalar.activation(
                                    zt[:], gxv[:, :, t, :], Act.Copy)
                                hprev = hall[:, (t - 1) * 16:t * 16]
                                for gb in range(4):
                                    nc.tensor.matmul(
                                        zt[:, gb * 16:(gb + 1) * 16],
                                        whh_s[:, l * 512 + gb * 128:
                                              l * 512 + (gb + 1) * 128],
                                        hprev,
                                        start=False, stop=True,
                                        skip_group_check=True)
                                nc.scalar.activation(
                                    sg[:, 0:48], zt[:, 0:48], Act.Sigmoid)
                                nc.scalar.activation(
                                    sg[:, 48:64], zt[:, 48:64], Act.Tanh)

                            i_sl = sg[:, 0:16]
                            f_sl = sg[:, 16:32]
                            o_sl = sg[:, 32:48]
                            tg_sl = sg[:, 48:64]

                            mt = ew.tile([128, 16], f32, tag="mt")
                            nc.vector.tensor_tensor(mt[:], i_sl, tg_sl,
                                                    Alu.mult)
                            if t == 0:
                                nc.vector.tensor_copy(cst[:], mt[:])
                            else:
                                nc.vector.tensor_tensor(
                                    cst[:], cst[:], f_sl, Alu.mult)
                                nc.vector.tensor_tensor(
                                    cst[:], cst[:], mt[:], Alu.add)
                            th = ew.tile([128, 16], f32, tag="th")
                            nc.scalar.activation(th[:], cst[:], Act.Tanh)
                            nc.vector.tensor_tensor(
                                hall[:, t * 16:(t + 1) * 16], th[:], o_sl,
                                Alu.mult)

                        hall_prev = hall

                  # ---------------- phase C --------------------------------
                  with tc.tile_pool(name=f"pcp{r}", bufs=1) as pcp:
                        wout_s = pcp.tile([128, OUTP], f32, tag="wout",
                                          name=f"wout_s{r}")
                        padd_s = pcp.tile([16, OUTP], f32, tag="padd",
                                          name=f"padd_s{r}")
                        nc.sync.dma_start(out=wout_s[:], in_=woutT[:])
                        nc.sync.dma_start(out=padd_s[:], in_=padd[:])
                        msb = pcp.tile([128, OUTP], f32, tag="msb",
                                       name=f"msb{r}")
                        with tc.tile_pool(name=f"psc{r}", bufs=2,
                                          space="PSUM") as psc:
                            for j in range(4):
                                lhs = hall_prev[:, j * 128:(j + 1) * 128]
                                for ob in range(OUTP // 512):
                                    pc = psc.tile([128, 512], f32, tag="pc")
                                    nc.tensor.matmul(
                                        pc[:], lhs,
                                        wout_s[:, ob * 512:(ob + 1) * 512],
                                        start=True, stop=True,
                                        skip_group_check=True)
                                    if j == 0:
                                        nc.vector.tensor_copy(
                                            msb[:, ob * 512:(ob + 1) * 512],
                                            pc[:])
                                    else:
                                        nc.vector.tensor_tensor(
                                            msb[:, ob * 512:(ob + 1) * 512],
                                            pc[:],
                                            msb[:, ob * 512:(ob + 1) * 512],
                                            Alu.max)
                        f1 = pcp.tile([64, OUTP], f32, tag="f1",
                                      name=f"f1{r}")
                        fs = pcp.tile([64, OUTP], f32, tag="fs",
                                      name=f"fs{r}")
                        nc.sync.dma_start(out=fs[:], in_=msb[64:128, :])
                        nc.vector.tensor_tensor(
                            f1[:], msb[0:64, :], fs[:], Alu.max)
                        nc.sync.dma_start(out=fs[0:32, :], in_=f1[32:64, :])
                        nc.vector.tensor_tensor(
                            f1[0:32, :], f1[0:32, :], fs[0:32, :], Alu.max)
                        nc.sync.dma_start(out=fs[0:16, :], in_=f1[16:32, :])
                        nc.vector.tensor_tensor(
                            f1[0:16, :], f1[0:16, :], fs[0:16, :], Alu.max)
                        nc.vector.tensor_tensor(
                            f1[0:16, :], f1[0:16, :], padd_s[:], Alu.add)
                        nc.sync.dma_start(out=outp[:], in_=f1[0:16, :])

    nc.compile()
    return nc


def prep_inputs(x, bn_gamma, bn_beta, W_ih0, W_ih, W_hh, b_ih, b_hh,
                W_out, b_out, prior):
    """Host-side sharding / layout prep. Returns in_maps list."""
    x = np.asarray(x, np.float32)
    pad = INP - IN
    # time-major tokens: [B,T,IN] -> [T,B,IN] -> [TOK, INP] -> transpose
    xtb = np.ascontiguousarray(x.transpose(1, 0, 2).reshape(TOK, IN))
    xtb = np.pad(xtb, ((0, 0), (0, pad)))
    xT_full = np.ascontiguousarray(xtb.T)             # [INP, TOK]

    # gate rows reordered from torch (i,f,g,o) to kernel (i,f,o,g)
    PERM = np.r_[0:2 * H, 3 * H:4 * H, 2 * H:3 * H]

    W0 = np.asarray(W_ih0, np.float32)[PERM, :]
    w0T_full = np.ascontiguousarray(np.pad(W0, ((0, 0), (0, pad))).T)

    gp = np.pad(np.asarray(bn_gamma, np.float32), (0, pad))
    bp = np.pad(np.asarray(bn_beta, np.float32), (0, pad))

    bias = (np.asarray(b_ih, np.float32)
            + np.asarray(b_hh, np.float32))[:, PERM]  # [L, 512]
    Wih = np.asarray(W_ih, np.float32)[:, PERM, :]    # [L-1, 512, 128]
    Whh = np.asarray(W_hh, np.float32)[:, PERM, :]    # [L, 512, 128]

    wih_all = np.concatenate(
        [Wih[l].T for l in range(L - 1)], axis=1)     # [128, 15*512]
    whh_all = np.concatenate(
        [Whh[l].T for l in range(L)], axis=1)         # [128, 16*512]
    bias_all = np.stack(
        [bias[l].reshape(4, 128).T for l in range(L)],
        axis=1).reshape(128, L * 4)                   # [128, l*4+gb]

    WoT = np.zeros((128, OUTP), np.float32)
    WoT[:, :OUT] = np.asarray(W_out, np.float32).T

    p = np.clip(np.asarray(prior, np.float64), 1e-8, 1 - 1e-8)
    logit = (np.log(p) - np.log1p(-p)).astype(np.float32)
    paddv = np.zeros((16, OUTP), np.float32)
    paddv[:, :OUT] = np.asarray(b_out, np.float32)[None, :]
    paddv[:, 1:OUT] += logit[None, :]

    shared = {
        "wih_all": np.ascontiguousarray(wih_all),
        "whh_all": np.ascontiguousarray(whh_all),
        "bias_all": np.ascontiguousarray(bias_all),
        "woutT": WoT,
        "padd": paddv,
    }
    in_maps = []
    for c in range(NC):
        sl = slice(c * INL, (c + 1) * INL)
        # kernel's chunk ch places channel p*NCH + cc on partition p; the
        # wc stream reads w0T rows ch*128+p, so permute w0T rows to match.
        w0c = w0T_full[sl].reshape(128, NCH, 512).transpose(1, 0, 2)
        m = {
            "xT": np.ascontiguousarray(xT_full[sl]),
            "w0T": np.ascontiguousarray(w0c.reshape(INL, 512)),
            "gam": np.ascontiguousarray(gp[sl].reshape(128, NCH)),
            "bet": np.ascontiguousarray(bp[sl].reshape(128, NCH)),
        }
        m.update(shared)
        in_maps.append(m)
    return in_maps


_CACHED = {}


def kernel(**inputs):
    from concourse.bass_utils import run_bass_kernel_spmd

    if "nc" not in _CACHED:
        _CACHED["nc"] = build_kernel()
    nc = _CACHED["nc"]
    in_maps = prep_inputs(**inputs)
    res = run_bass_kernel_spmd(nc, in_maps, core_ids=list(range(NC)))
    _CACHED["res"] = res
    out = res.results[0]["outp"][:, :OUT]
    return np.ascontiguousarray(out)


if __name__ == "__main__":
    import reference
    inputs = {k: np.asarray(v) for k, v in reference.setup_inputs().items()}
    got = kernel(**inputs)
    exp = np.asarray(reference.reference(**inputs))
    denom = np.abs(exp).max() + 1e-9
    print("Relative error:", np.abs(got - exp).max() / denom)


# revision 28
# speedup vs baseline: 1.2873x; 1.2873x over previous
"""PriorLSTM Trainium2 kernel (8 NeuronCores, SPMD).

Model: BatchNorm1d(IN) -> 16-layer LSTM(H=128) -> Linear(H->OUT) -> max over T
       -> + prior logits.   B=16, T=32, IN=52686, OUT=2976.

Strategy:
  Phase A (tensor-parallel on IN): each core owns 6656 channels (padded).
    BN folded to per-channel scale/shift; big GEMM gx0.T[g,tok] accumulated
    over 52 K-chunks of 128 channels; one AllReduce of the [128,2048]
    partial gives every core the full layer-0 input projection.
  Phase B (replicated, zero collectives): every core runs the whole
    16-layer LSTM over all 16 batch samples. The per-step gate matmul
    [128x128]x[128,16] is weight-load dominated, so 16 samples cost the
    same as 2 -- replication buys out all cross-core traffic.
    States transposed: h.T/c.T = [128 h-dim, 16 batch]; weights are
    pre-transposed host-side so gates come out as gates.T [128,16] per
    gate block (order i,f,o,g).  The ACT engine preloads gx+bias into
    PSUM and the Whh matmuls accumulate onto it (start=False).
  Phase C: output projection + temporal max-pool; b_out and prior logits
    folded host-side into one additive constant.  Identical on all
    cores; core 0's output is returned.

The whole body (A+B+C) can be repeated `repeat` times inside one NEFF;
every pass recomputes identical values (used for slope-based timing of
true device execution, since the axon tunnel RTT ~85ms swamps wall
clock).  Tokens are time-major: tok = t*16 + b.
"""

import numpy as np

B, T, IN, H, L, OUT = 16, 32, 52686, 128, 16, 2976
EPS = 1e-5
NC = 8
INL = 6656          # channels per core (padded)
NCH = INL // 128    # 52 K-chunks per core
INP = INL * NC      # 53248
TOK = B * T         # 512
OUTP = 3072         # padded OUT


def build_kernel(sim=False, repeat=1):
    import concourse.bass as bass
    import concourse.bacc as bacc
    import concourse.mybir as mybir
    import concourse.tile as tile

    f32 = mybir.dt.float32
    Alu = mybir.AluOpType
    Act = mybir.ActivationFunctionType

    nc = bacc.Bacc(None, num_devices=1 if sim else NC)

    # ---------------- inputs ------------------------------------------------
    xT = nc.dram_tensor("xT", [INL, TOK], f32, kind="ExternalInput")
    w0T = nc.dram_tensor("w0T", [INL, 512], f32, kind="ExternalInput")
    gam = nc.dram_tensor("gam", [128, NCH], f32, kind="ExternalInput")
    bet = nc.dram_tensor("bet", [128, NCH], f32, kind="ExternalInput")
    # wih_all[p, (l-1)*512 + m]: W_ih[l].T for layers 1..15 (g rows x2)
    wih_all = nc.dram_tensor("wih_all", [128, (L - 1) * 512], f32,
                             kind="ExternalInput")
    # whh_all[p, l*512 + m]: W_hh[l].T for layers 0..15 (g rows x2)
    whh_all = nc.dram_tensor("whh_all", [128, L * 512], f32,
                             kind="ExternalInput")
    # bias_all[p, l*4 + gb] = (b_ih+b_hh)[l][gb*128+p] (g block x2)
    bias_all = nc.dram_tensor("bias_all", [128, 4 * L], f32,
                              kind="ExternalInput")
    woutT = nc.dram_tensor("woutT", [128, OUTP], f32, kind="ExternalInput")
    padd = nc.dram_tensor("padd", [16, OUTP], f32, kind="ExternalInput")

    outp = nc.dram_tensor("outp", [16, OUTP], f32, kind="ExternalOutput")

    with tile.TileContext(nc) as tc:
        with (
            tc.tile_pool(name="big", bufs=1) as big,
            tc.tile_pool(name="wstream", bufs=3) as wst,
            tc.tile_pool(name="small", bufs=2) as small,
            tc.tile_pool(name="ew", bufs=3) as ew,
            tc.tile_pool(name="dram", bufs=2, space="DRAM") as dpool,
        ):
            # persistent loads (once per NEFF)
            sums = big.tile([128, NCH], f32, tag="sums")
            sumsq = big.tile([128, NCH], f32, tag="sumsq")
            gams = big.tile([128, NCH], f32, tag="gams")
            bets = big.tile([128, NCH], f32, tag="bets")
            nc.sync.dma_start(out=gams[:], in_=gam[:])
            nc.sync.dma_start(out=bets[:], in_=bet[:])

            wih_s = big.tile([128, (L - 1) * 512], f32, tag="wih")
            whh_s = big.tile([128, L * 512], f32, tag="whh")
            ball_s = big.tile([128, 4 * L], f32, tag="ball")
            nc.sync.dma_start(out=wih_s[:], in_=wih_all[:])
            nc.sync.dma_start(out=whh_s[:], in_=whh_all[:])
            nc.sync.dma_start(out=ball_s[:], in_=bias_all[:])

            gx0 = big.tile([128, 4 * TOK], f32, tag="gx0")
            nc.vector.memset(gx0[:], 0.0)
            # channel ch = p*NCH + c: per-partition reads are contiguous
            xview = xT.rearrange("(p c) t -> p c t", c=NCH)

            for r in range(repeat):
                # ---------------- phase A ----------------------------------
                _a = 0
                with tc.tile_pool(name=f"xtsp{r}_{_a}", bufs=1) as xtsp:
                    SLAB = 13
                    xts_l = [xtsp.tile([128, SLAB * TOK], f32, tag=f"xts{s}",
                                       name=f"xts{r}_{_a}_{s}") for s in range(4)]

                    def xchunk(ch):
                        return xts_l[ch // SLAB][
                            :, (ch % SLAB) * TOK:(ch % SLAB + 1) * TOK]

                    for s in range(4):
                        nc.sync.dma_start(
                            out=xts_l[s][:].rearrange(
                                "p (c t) -> p c t", t=TOK),
                            in_=xview[:, s * SLAB:(s + 1) * SLAB, :],
                        )
                    for ch in range(NCH):
                        xc = xchunk(ch)
                        scr = small.tile([128, TOK], f32, tag="scr")
                        nc.vector.tensor_reduce(
                            sums[:, ch:ch + 1], xc,
                            mybir.AxisListType.X, Alu.add,
                        )
                        nc.vector.scalar_tensor_tensor(
                            out=scr[:], in0=xc, scalar=1.0, in1=xc,
                            op0=Alu.mult, op1=Alu.mult,
                            accum_out=sumsq[:, ch:ch + 1],
                        )
                    mean = big.tile([128, NCH], f32, tag="mean")
                    var = big.tile([128, NCH], f32, tag="var")
                    sd = big.tile([128, NCH], f32, tag="sd")
                    r0 = big.tile([128, NCH], f32, tag="r0")
                    t1 = big.tile([128, NCH], f32, tag="t1")
                    scl = big.tile([128, NCH], f32, tag="scl")
                    b2 = big.tile([128, NCH], f32, tag="b2")
                    nc.vector.tensor_scalar_mul(mean[:], sums[:], 1.0 / TOK)
                    nc.vector.tensor_scalar_mul(var[:], sumsq[:], 1.0 / TOK)
                    nc.vector.scalar_tensor_tensor(
                        out=t1[:], in0=mean[:], scalar=-1.0, in1=mean[:],
                        op0=Alu.mult, op1=Alu.mult)
                    nc.vector.tensor_tensor(var[:], var[:], t1[:], Alu.add)
                    nc.vector.tensor_scalar_add(var[:], var[:], EPS)
                    nc.scalar.activation(sd[:], var[:], Act.Sqrt)
                    nc.vector.reciprocal(r0[:], sd[:])
                    nc.vector.tensor_tensor(t1[:], r0[:], r0[:], Alu.mult)
                    nc.vector.tensor_tensor(t1[:], t1[:], var[:], Alu.mult)
                    nc.vector.tensor_scalar(
                        out=t1[:], in0=t1[:], scalar1=-0.5, scalar2=1.5,
                        op0=Alu.mult, op1=Alu.add)
                    nc.vector.tensor_tensor(r0[:], r0[:], t1[:], Alu.mult)
                    nc.vector.tensor_tensor(scl[:], r0[:], gams[:], Alu.mult)
                    nc.vector.scalar_tensor_tensor(
                        out=b2[:], in0=mean[:], scalar=-1.0, in1=scl[:],
                        op0=Alu.mult, op1=Alu.mult)
                    nc.vector.tensor_tensor(b2[:], b2[:], bets[:], Alu.add)

                    with tc.tile_pool(name=f"psgx{r}_{_a}", bufs=1,
                                      space="PSUM") as psgx:
                        gx = [psgx.tile([128, TOK], f32, name=f"gx{r}_{_a}_{gb}",
                                        tag=f"gx{gb}") for gb in range(4)]
                        for ch in range(NCH):
                            xc = xchunk(ch)
                            nc.vector.tensor_scalar(
                                out=xc, in0=xc,
                                scalar1=scl[:, ch:ch + 1],
                                scalar2=b2[:, ch:ch + 1],
                                op0=Alu.mult, op1=Alu.add)
                            wc = wst.tile([128, 512], f32, tag="wc")
                            nc.sync.dma_start(
                                out=wc[:], in_=w0T[ch * 128:(ch + 1) * 128, :])
                            for gb in range(4):
                                nc.tensor.matmul(
                                    gx[gb][:], wc[:, gb * 128:(gb + 1) * 128],
                                    xc,
                                    start=(ch == 0), stop=(ch == NCH - 1),
                                    skip_group_check=True)

                        for gb in range(4):
                            nc.vector.tensor_copy(
                                gx0[:, gb * TOK:(gb + 1) * TOK], gx[gb][:])

                arin = dpool.tile([128, 4 * TOK], f32, tag="arin")
                arout = dpool.tile([128, 4 * TOK], f32, tag="arout")
                nc.gpsimd.dma_start(out=arin[:], in_=gx0[:])
                if sim:
                    nc.gpsimd.dma_start(out=arout[:], in_=arin[:])
                else:
                    nc.gpsimd.collective_compute(
                        "AllReduce", Alu.add,
                        ins=[arin[:].opt()], outs=[arout[:].opt()],
                        replica_groups=[list(range(NC))],
                    )
                nc.sync.dma_start(out=gx0[:], in_=arout[:])

                # ---------------- phase B ----------------------------------
                cst = big.tile([128, 16], f32, tag="cst")
                hall_prev = None
                with tc.tile_pool(name=f"hp{r}", bufs=2) as hp:
                  with (
                    tc.tile_pool(name=f"gxp{r}", bufs=2) as gxp,
                    tc.tile_pool(name=f"psg2{r}", bufs=1,
                                 space="PSUM") as psg2,
                    tc.tile_pool(name=f"psz{r}", bufs=4, space="PSUM") as psz,
                  ):
                    for l in range(L):
                        gxl = gxp.tile([128, 4 * TOK], f32, tag="gxl")
                        if l == 0:
                            for gb in range(4):
                                nc.vector.tensor_scalar_add(
                                    gxl[:, gb * TOK:(gb + 1) * TOK],
                                    gx0[:, gb * TOK:(gb + 1) * TOK],
                                    ball_s[:, l * 4 + gb:l * 4 + gb + 1])
                        else:
                            for gb in range(4):
                                pg = psg2.tile([128, TOK], f32, tag=f"pg{gb}")
                                nc.tensor.matmul(
                                    pg[:],
                                    wih_s[:, (l - 1) * 512 + gb * 128:
                                          (l - 1) * 512 + (gb + 1) * 128],
                                    hall_prev[:],
                                    start=True, stop=True,
                                    skip_group_check=True)
                                nc.vector.tensor_scalar_add(
                                    gxl[:, gb * TOK:(gb + 1) * TOK], pg[:],
                                    ball_s[:, l * 4 + gb:l * 4 + gb + 1])
                        gxv = gxl[:, :].rearrange("p (g t b) -> p g t b",
                                                  g=4, b=16)

                        hall = hp.tile([128, TOK], f32, tag="hall")

                        # gate blocks host-reordered to (i, f, o, g)
                        for t in range(T):
                            sg = ew.tile([128, 64], f32, tag="sg")
                            if t == 0:
                                nc.scalar.activation(
                                    sg[:, 0:48], gxv[:, 0:3, 0, :],
                                    Act.Sigmoid)
                                nc.scalar.activation(
                                    sg[:, 48:64], gxv[:, 3:4, 0, :],
                                    Act.Tanh)
                            else:
                                zt = psz.tile([128, 64], f32, tag="zt")
                                # ACT preloads gx+bias; matmuls accumulate
                                nc.scalar.activation(
                                    zt[:], gxv[:, :, t, :], Act.Copy)
                                hprev = hall[:, (t - 1) * 16:t * 16]
                                for gb in range(4):
                                    nc.tensor.matmul(
                                        zt[:, gb * 16:(gb + 1) * 16],
                                        whh_s[:, l * 512 + gb * 128:
                                              l * 512 + (gb + 1) * 128],
                                        hprev,
                                        start=False, stop=True,
                                        skip_group_check=True)
                                nc.scalar.activation(
                                    sg[:, 0:48], zt[:, 0:48], Act.Sigmoid)
                                nc.scalar.activation(
                                    sg[:, 48:64], zt[:, 48:64], Act.Tanh)

                            i_sl = sg[:, 0:16]
                            f_sl = sg[:, 16:32]
                            o_sl = sg[:, 32:48]
                            tg_sl = sg[:, 48:64]

                            mt = ew.tile([128, 16], f32, tag="mt")
                            nc.vector.tensor_tensor(mt[:], i_sl, tg_sl,
                                                    Alu.mult)
                            if t == 0:
                                nc.vector.tensor_copy(cst[:], mt[:])
                            else:
                                nc.vector.tensor_tensor(
                                    cst[:], cst[:], f_sl, Alu.mult)
                                nc.vector.tensor_tensor(
                                    cst[:], cst[:], mt[:], Alu.add)
                            th = ew.tile([128, 16], f32, tag="th")
                            nc.scalar.activation(th[:], cst[:], Act.Tanh)
                            nc.vector.tensor_tensor(
                                hall[:, t * 16:(t + 1) * 16], th[:], o_sl,
                                Alu.mult)

                        hall_prev = hall

                  # ---------------- phase C --------------------------------
                  with tc.tile_pool(name=f"pcp{r}", bufs=1) as pcp:
                        wout_s = pcp.tile([128, OUTP], f32, tag="wout",
                                          name=f"wout_s{r}")
                        padd_s = pcp.tile([16, OUTP], f32, tag="padd",
                                          name=f"padd_s{r}")
                        nc.sync.dma_start(out=wout_s[:], in_=woutT[:])
                        nc.sync.dma_start(out=padd_s[:], in_=padd[:])
                        msb = pcp.tile([128, OUTP], f32, tag="msb",
                                       name=f"msb{r}")
                        with tc.tile_pool(name=f"psc{r}", bufs=2,
                                          space="PSUM") as psc:
                            for j in range(4):
                                lhs = hall_prev[:, j * 128:(j + 1) * 128]
                                for ob in range(OUTP // 512):
                                    pc = psc.tile([128, 512], f32, tag="pc")
                                    nc.tensor.matmul(
                                        pc[:], lhs,
                                        wout_s[:, ob * 512:(ob + 1) * 512],
                                        start=True, stop=True,
                                        skip_group_check=True)
                                    if j == 0:
                                        nc.vector.tensor_copy(
                                            msb[:, ob * 512:(ob + 1) * 512],
                                            pc[:])
                                    else:
                                        nc.vector.tensor_tensor(
                                            msb[:, ob * 512:(ob + 1) * 512],
                                            pc[:],
                                            msb[:, ob * 512:(ob + 1) * 512],
                                            Alu.max)
                        f1 = pcp.tile([64, OUTP], f32, tag="f1",
                                      name=f"f1{r}")
                        fs = pcp.tile([64, OUTP], f32, tag="fs",
                                      name=f"fs{r}")
                        nc.sync.dma_start(out=fs[:], in_=msb[64:128, :])
                        nc.vector.tensor_tensor(
                            f1[:], msb[0:64, :], fs[:], Alu.max)
                        nc.sync.dma_start(out=fs[0:32, :], in_=f1[32:64, :])
                        nc.vector.tensor_tensor(
                            f1[0:32, :], f1[0:32, :], fs[0:32, :], Alu.max)
                        nc.sync.dma_start(out=fs[0:16, :], in_=f1[16:32, :])
                        nc.vector.tensor_tensor(
                            f1[0:16, :], f1[0:16, :], fs[0:16, :], Alu.max)
                        nc.vector.tensor_tensor(
                            f1[0:16, :], f1[0:16, :], padd_s[:], Alu.add)
                        nc.sync.dma_start(out=outp[:], in_=f1[0:16, :])

    nc.compile()
    return nc


def prep_inputs(x, bn_gamma, bn_beta, W_ih0, W_ih, W_hh, b_ih, b_hh,
                W_out, b_out, prior):
    """Host-side sharding / layout prep. Returns in_maps list."""
    x = np.asarray(x, np.float32)
    pad = INP - IN
    # time-major tokens: [B,T,IN] -> [T,B,IN] -> [TOK, INP] -> transpose
    xtb = np.ascontiguousarray(x.transpose(1, 0, 2).reshape(TOK, IN))
    xtb = np.pad(xtb, ((0, 0), (0, pad)))
    xT_full = np.ascontiguousarray(xtb.T)             # [INP, TOK]

    # gate rows reordered from torch (i,f,g,o) to kernel (i,f,o,g)
    PERM = np.r_[0:2 * H, 3 * H:4 * H, 2 * H:3 * H]

    W0 = np.asarray(W_ih0, np.float32)[PERM, :]
    w0T_full = np.ascontiguousarray(np.pad(W0, ((0, 0), (0, pad))).T)

    gp = np.pad(np.asarray(bn_gamma, np.float32), (0, pad))
    bp = np.pad(np.asarray(bn_beta, np.float32), (0, pad))

    bias = (np.asarray(b_ih, np.float32)
            + np.asarray(b_hh, np.float32))[:, PERM]  # [L, 512]
    Wih = np.asarray(W_ih, np.float32)[:, PERM, :]    # [L-1, 512, 128]
    Whh = np.asarray(W_hh, np.float32)[:, PERM, :]    # [L, 512, 128]

    wih_all = np.concatenate(
        [Wih[l].T for l in range(L - 1)], axis=1)     # [128, 15*512]
    whh_all = np.concatenate(
        [Whh[l].T for l in range(L)], axis=1)         # [128, 16*512]
    bias_all = np.stack(
        [bias[l].reshape(4, 128).T for l in range(L)],
        axis=1).reshape(128, L * 4)                   # [128, l*4+gb]

    WoT = np.zeros((128, OUTP), np.float32)
    WoT[:, :OUT] = np.asarray(W_out, np.float32).T

    p = np.clip(np.asarray(prior, np.float64), 1e-8, 1 - 1e-8)
    logit = (np.log(p) - np.log1p(-p)).astype(np.float32)
    paddv = np.zeros((16, OUTP), np.float32)
    paddv[:, :OUT] = np.asarray(b_out, np.float32)[None, :]
    paddv[:, 1:OUT] += logit[None, :]

    shared = {
        "wih_all": np.ascontiguousarray(wih_all),
        "whh_all": np.ascontiguousarray(whh_all),
        "bias_all": np.ascontiguousarray(bias_all),
        "woutT": WoT,
        "padd": paddv,
    }
    in_maps = []
    for c in range(NC):
        sl = slice(c * INL, (c + 1) * INL)
        # kernel's chunk ch places channel p*NCH + cc on partition p; the
        # wc stream reads w0T rows ch*128+p, so permute w0T rows to match.
        w0c = w0T_full[sl].reshape(128, NCH, 512).transpose(1, 0, 2)
        m = {
            "xT": np.ascontiguousarray(xT_full[sl]),
            "w0T": np.ascontiguousarray(w0c.reshape(INL, 512)),
            "gam": np.ascontiguousarray(gp[sl].reshape(128, NCH)),
            "bet": np.ascontiguousarray(bp[sl].reshape(128, NCH)),
        }
        m.update(shared)
        in_maps.append(m)
    return in_maps


_CACHED = {}


def kernel(**inputs):
    from concourse.bass_utils import run_bass_kernel_spmd

    if "nc" not in _CACHED:
        _CACHED["nc"] = build_kernel()
    nc = _CACHED["nc"]
    in_maps = prep_inputs(**inputs)
    res = run_bass_kernel_spmd(nc, in_maps, core_ids=list(range(NC)))
    _CACHED["res"] = res
    out = res.results[0]["outp"][:, :OUT]
    return np.ascontiguousarray(out)


if __name__ == "__main__":
    import reference
    inputs = {k: np.asarray(v) for k, v in reference.setup_inputs().items()}
    got = kernel(**inputs)
    exp = np.asarray(reference.reference(**inputs))
    denom = np.abs(exp).max() + 1e-9
    print("Relative error:", np.abs(got - exp).max() / denom)
